# revision 19
# baseline (speedup 1.0000x reference)
"""LocalGOCor (PWC-Net local correlation, radius 4) on 8 Trainium2 NeuronCores.

scores[b, d, y, x] = gain * sum_c f[b,c,y,x] * q_zeropad[b, c, y+dy, x+dx]
for d = dy*9+dx, dy/dx in [0,9)  (displacement dy-4, dx-4).

v2 strategy (data-parallel over batch, 2 samples per core). The kernel is
HBM-bandwidth-bound, so every stage is built to minimize DMA bytes:

  - Inputs are quantized to int8 on the host (scale 31.75 ~ clip at 4
    sigma); on-device DVE upcasts them to bf16 holding EXACT integers in
    [-127,127], so the TensorE matmuls and fp32 PSUM accumulation are
    integer-exact (products <= 127^2, sums < 2^24).  The ~4k clipped
    elements are corrected exactly on the host.  Input HBM traffic halves
    vs bf16 (16.8 -> 8.4 MB/core).
  - Image tiled into 8x8 pixel blocks.  Per block one TensorE matmul:
    lhsT = F[c, 64 pixels], rhs = Q window [c, <=16y, <=16x] read from a
    whole-sample Q tile.  PSUM[p=(ph,ys,xs), (m, wy, wx)] for a whole
    64-row-strip of 16 blocks lives in one 4-bank tile; two blocks share
    each PE column half via tile_position (0,0)/(0,64).
  - Drains (PSUM f32 -> int8, x runtime scale c = gain*127/(75*S^2)) are
    monolithic per strip (full 128-partition efficiency) but write an
    [s, wy, m, wx]-ordered SBUF tile via a stride-permuted AP.  The
    output DMA then ships, per (ph, ys) partition group, only window
    rows [ys, ys+9) - 144 B/pixel instead of 256 - in contiguous 1152-B
    runs.  Output HBM traffic: 8.39 -> 4.72 MB/core.
  - Drain engine split ACT:DVE = 26:6 strips balances ACT (drain-only)
    against DVE (drains + all int8->bf16 upcasts at 2x).  Output DMAs go
    on the otherwise-idle GPSIMD/SWDGE queue so they never block the
    HWDGE input rings (q on SP, f on ACT).
  - Host unshard: zero-copy as_strided shear + descale + sparse exact
    clip corrections + zeroing of out-of-image displacement stripes.
"""

import numpy as np

B, C, H, W = 16, 128, 128, 128
R = 4
ND = 2 * R + 1            # 9 displacements per axis
NCORES = 8
BLOC = B // NCORES        # 2 samples per core
BY, BX = 8, 8             # pixels per block -> M = 64
WY, WX = BY + 2 * R, BX + 2 * R   # 16, 16 query window
NWIN = WY * WX            # 256
YBLK = 64                 # image rows per output chunk
NYC = H // YBLK           # 2
NYSUB = YBLK // BY        # 8 y-strips per chunk
NXB = W // BX             # 16 x-blocks
S = np.float32(31.75)     # int8 units per 1.0 of raw f/q
ODEN = np.float32(75.0)   # int8 output covers scores in [-ODEN, ODEN]
# drain-engine split: strips [0, NA) on ACT, [NA, 8) on DVE, per chunk
NA_PER_CHUNK = [7, 7, 6, 6]
# True: ship only window rows [ys, ys+9) per (ph, ys) partition group
# (144 B/pixel, 16 DMAs per wave); False: dense 256 B/pixel, 1 DMA/wave
SLICED = True

_CACHE = {}


def _build():
    import concourse.bacc as bacc
    import concourse.tile as tile
    import concourse.mybir as mybir
    from contextlib import ExitStack

    nc = bacc.Bacc(
        "TRN2",
        target_bir_lowering=False,
        debug=False,
        enable_asserts=False,
        num_devices=NCORES,
    )
    f32 = mybir.dt.float32
    bf16 = mybir.dt.bfloat16
    i8 = mybir.dt.int8

    f_dram = nc.dram_tensor("f8", [BLOC, C, NYC, NYSUB * NXB, BY * BX], i8,
                            kind="ExternalInput").ap()
    q_dram = nc.dram_tensor("q8", [BLOC, C, H, W], i8, kind="ExternalInput").ap()
    c_dram = nc.dram_tensor("csc", [C, 1], f32, kind="ExternalInput").ap()
    if SLICED:
        # [b, ph, ys, xs, s(16), wy(9), m, wx] : per (ph,ys) DMA the dst
        # block [xs, s-range, 9, 8, 16] is per-partition contiguous;
        # s = yc*8 + y0i
        o_dram = nc.dram_tensor(
            "out", [BLOC, 2, BY, BX, NYC * NYSUB, ND, 8, WX], i8,
            kind="ExternalOutput").ap()
    else:
        # dense: [b, yc, p(128), s(8), m, wwin(256)]
        o_dram = nc.dram_tensor(
            "out", [BLOC, NYC, C, NYSUB, 8, NWIN], i8,
            kind="ExternalOutput").ap()

    QP = [(0, 12), (12, 68), (68, H)]       # q load pieces (rows)
    FP = [(0, 32), (32, 80), (80, 128)]     # f load pieces (block-rows)

    with tile.TileContext(nc) as tc, ExitStack() as ctx:
        qspool = ctx.enter_context(tc.tile_pool(name="qspool", bufs=1))
        qpool = ctx.enter_context(tc.tile_pool(name="qpool", bufs=2))
        fspool = ctx.enter_context(tc.tile_pool(name="fspool", bufs=2))
        fpool = ctx.enter_context(tc.tile_pool(name="fpool", bufs=2))
        opool = ctx.enter_context(tc.tile_pool(name="opool", bufs=2))
        cpool = ctx.enter_context(tc.tile_pool(name="cpool", bufs=1))
        pspool = ctx.enter_context(tc.tile_pool(name="pspool", bufs=2, space="PSUM"))

        ct = cpool.tile([C, 1], f32, tag="ct")
        nc.sync.dma_start(out=ct[:, :], in_=c_dram[:, :])
        c_ap = ct[:, 0:1]

        # ---- input staging helpers ------------------------------------
        def load_q_piece(b, ql, pi):
            lo, hi = QP[pi]
            qs = qspool.tile([C, hi - lo, W], i8, tag=f"qs{pi}")
            nc.sync.dma_start(out=qs[:, :, :], in_=q_dram[b, :, lo:hi, :])
            return qs

        def up_q_piece(ql, qs, pi):
            lo, hi = QP[pi]
            nc.vector.tensor_scalar_mul(ql[:, lo:hi, :], qs[:, :, :], 1.0)

        def load_f(b, yc):
            fs = fspool.tile([C, NYSUB * NXB, BY * BX], i8, tag="fs")
            for lo, hi in FP:
                nc.scalar.dma_start(out=fs[:, lo:hi, :],
                                    in_=f_dram[b, :, yc, lo:hi, :])
            return fs

        def up_f(fs):
            ft = fpool.tile([C, NYSUB * NXB, BY * BX], bf16, tag="ft")
            for lo, hi in FP:
                nc.vector.tensor_scalar_mul(ft[:, lo:hi, :], fs[:, lo:hi, :], 1.0)
            return ft

        # ---- preamble -------------------------------------------------
        ql0 = qpool.tile([C, H, W], bf16, tag="ql")
        qs00 = load_q_piece(0, ql0, 0)
        qs01 = load_q_piece(0, ql0, 1)
        qs02 = load_q_piece(0, ql0, 2)
        up_q_piece(ql0, qs00, 0)
        up_q_piece(ql0, qs01, 1)
        chunks = [(b, yc) for b in range(BLOC) for yc in range(NYC)]
        fs_next = load_f(*chunks[0])
        ft_next = up_f(fs_next)
        up_q_piece(ql0, qs02, 2)
        qls = [ql0]

        ot = None
        for ci, (b, yc) in enumerate(chunks):
            ql = qls[b]
            ft = ft_next
            na = NA_PER_CHUNK[ci]
            if SLICED and yc == 0:
                # one shared output tile per sample; ACT and DVE drain
                # disjoint slots, one DMA range covers both
                ot = opool.tile([C, NYC * NYSUB, WY, 8, WX], i8, tag="ot")
            elif not SLICED:
                ot = opool.tile([C, NYSUB, 8, NWIN], i8, tag="ot")

            for y0i in range(NYSUB):
                r_lo = yc * YBLK + y0i * BY - R
                rl, rh = max(r_lo, 0), min(r_lo + WY, H)
                py = rl - r_lo
                pt = pspool.tile([C, 8, WY, WX], f32, tag="pt")
                for m in range(8):
                    for ph in range(2):
                        jx = ph * 8 + m
                        blk = y0i * NXB + jx
                        c_lo = BX * jx - R
                        cl, ch = max(c_lo, 0), min(c_lo + WX, W)
                        px = cl - c_lo
                        nc.tensor.matmul(
                            pt[64 * ph:64 * ph + 64, m,
                               py:py + (rh - rl), px:px + (ch - cl)],
                            ft[:, blk, :],
                            ql[:, rl:rh, cl:ch],
                            start=True, stop=True,
                            tile_position=(0, 64 * ph),
                        )

                # schedule hooks: prefetch next chunk / next sample
                if y0i == 0 and ci + 1 < len(chunks):
                    fs_next = load_f(*chunks[ci + 1])
                    ft_next = up_f(fs_next)
                if ci == 0 and y0i == 2:
                    ql1 = qpool.tile([C, H, W], bf16, tag="ql")
                    qs10 = load_q_piece(1, ql1, 0)
                    qs11 = load_q_piece(1, ql1, 1)
                    up_q_piece(ql1, qs10, 0)
                    up_q_piece(ql1, qs11, 1)
                    qls.append(ql1)
                if ci == 0 and y0i == 4:
                    # last q issue: keeps the SP HWDGE ring all-inputs
                    # before the first output DMA (FIFO per ring)
                    qs12 = load_q_piece(1, qls[1], 2)
                    up_q_piece(qls[1], qs12, 2)

                # drain: PSUM f32 -> int8 with runtime scale, into the
                # [s, wy, m, wx]-ordered tile via a permuted-stride AP
                if SLICED:
                    slot = NYSUB * yc + y0i
                    dst = ot[:, slot, :, :, :].transpose([0, 2, 1, 3])
                else:
                    dst = ot[:, y0i, :, :]
                if y0i < na:
                    nc.scalar.mul(dst, pt[:, :, :, :], c_ap)
                else:
                    nc.vector.tensor_scalar_mul(dst, pt[:, :, :, :], c_ap)

                # output DMA waves on the SP HWDGE ring (cheap RTL
                # desc-gen; SWDGE costs ~1us Pool seq time per dma_start)
                if SLICED and y0i == NYSUB - 1:
                    s0, s1 = NYSUB * yc, NYSUB * yc + NYSUB
                    for ph in range(2):
                        for ys in range(BY):
                            p0 = 64 * ph + 8 * ys
                            nc.sync.dma_start(
                                out=o_dram[b, ph, ys, :, s0:s1],
                                in_=ot[p0:p0 + 8, s0:s1, ys:ys + ND, :, :])
                elif not SLICED and y0i % 4 == 3:
                    s0, s1 = y0i - 3, y0i + 1
                    nc.sync.dma_start(out=o_dram[b, yc, :, s0:s1, :, :],
                                      in_=ot[:, s0:s1, :, :])

    nc.compile()
    return nc


def _get_nc():
    if "nc" not in _CACHE:
        _CACHE["nc"] = _build()
    return _CACHE["nc"]


def _quant(x):
    r = np.rint(np.asarray(x, dtype=np.float32) * S)
    return np.clip(r, -127, 127).astype(np.int8)


def pack_f8(f):
    """[Bany, C, H, W] f32 -> int8 [Bany, C, NYC, NYSUB*NXB, BY*BX]."""
    v = _quant(f)
    n = v.shape[0]
    v = v.reshape(n, C, NYC, NYSUB, BY, NXB, BX)
    v = v.transpose(0, 1, 2, 3, 5, 4, 6)
    return np.ascontiguousarray(v.reshape(n, C, NYC, NYSUB * NXB, BY * BX))


def make_in_maps(f, q, gain):
    fp = pack_f8(f)
    qb = _quant(q)
    csc = np.full((C, 1), gain * 127.0 / (ODEN * S * S), dtype=np.float32)
    return [
        {"f8": fp[BLOC * c:BLOC * (c + 1)], "q8": qb[BLOC * c:BLOC * (c + 1)],
         "csc": csc}
        for c in range(NCORES)
    ]


def _clip_corrections(f, q, gain, est):
    """Add exact corrections for host-clipped int8 elements into est."""
    fr = np.rint(f * S)
    qr = np.rint(q * S)
    fcl = np.abs(fr) > 127
    qcl = np.abs(qr) > 127
    if not (fcl.any() or qcl.any()):
        return
    f8 = np.clip(fr, -127, 127) * (1.0 / S)
    q8 = np.clip(qr, -127, 127) * (1.0 / S)
    g = np.float32(gain)
    h, w = f.shape[2], f.shape[3]
    bi, ci, yi, xi = np.nonzero(fcl)
    qpad_t = np.pad(q, ((0, 0), (0, 0), (R, R), (R, R)))
    qpad_8 = np.pad(q8, ((0, 0), (0, 0), (R, R), (R, R)))
    for dy in range(ND):
        for dx in range(ND):
            d = dy * ND + dx
            tq = qpad_t[bi, ci, yi + dy, xi + dx]
            t8 = qpad_8[bi, ci, yi + dy, xi + dx]
            delta = f[bi, ci, yi, xi] * tq - f8[bi, ci, yi, xi] * t8
            np.add.at(est, (bi, np.full_like(bi, d), yi, xi), g * delta)
    bj, cj, yj, xj = np.nonzero(qcl)
    fpad_t = np.pad(f, ((0, 0), (0, 0), (R, R), (R, R)))
    fpad_8 = np.pad(f8, ((0, 0), (0, 0), (R, R), (R, R)))
    fpad_cl = np.pad(fcl, ((0, 0), (0, 0), (R, R), (R, R)))
    for dy in range(ND):
        for dx in range(ND):
            d = dy * ND + dx
            tf = fpad_t[bj, cj, yj + 2 * R - dy, xj + 2 * R - dx]
            t8 = fpad_8[bj, cj, yj + 2 * R - dy, xj + 2 * R - dx]
            already = fpad_cl[bj, cj, yj + 2 * R - dy, xj + 2 * R - dx]
            delta = np.where(already, 0.0,
                             tf * q[bj, cj, yj, xj] - t8 * q8[bj, cj, yj, xj])
            yy = yj + R - dy
            xx = xj + R - dx
            ok = (yy >= 0) & (yy < h) & (xx >= 0) & (xx < w)
            np.add.at(est, (bj[ok], np.full_like(bj[ok], d), yy[ok], xx[ok]),
                      (g * delta)[ok])


def _extract(O):
    """Device int8 output -> [B, 81, H, W] f32 (band shear via as_strided)."""
    Of = np.ascontiguousarray(O.astype(np.float32) * np.float32(ODEN / 127.0))
    if SLICED:
        # Of: [B, 2, BY, BX, NYC*NYSUB, ND, 8, WX]
        sb, sph, sys, sxs, ss, swy, sm, swx = Of.strides
        T = np.lib.stride_tricks.as_strided(
            Of,
            shape=(B, ND, ND, NYC * NYSUB, BY, 2, 8, BX),
            strides=(sb, swy, swx, ss, sys, sph, sm, sxs + swx),
        )
    else:
        # Of: [B, NYC, 2, BY, BX, NYSUB, 8, WY, WX]
        V = Of.reshape(B, NYC, 2, BY, BX, NYSUB, 8, WY, WX)
        sb, syc, sph, sys, sxs, ss, sm, swy, swx = V.strides
        T = np.lib.stride_tricks.as_strided(
            V,
            shape=(B, ND, ND, NYC, NYSUB, BY, 2, 8, BX),
            strides=(sb, swy, swx, syc, ss, sys + swy, sph, sm, sxs + swx),
        )
    out = np.ascontiguousarray(T.reshape(B, ND * ND, H, W))
    for dy in range(ND):
        for dx in range(ND):
            d = dy * ND + dx
            if dy < R:
                out[:, d, 0:R - dy, :] = 0.0
            elif dy > R:
                out[:, d, H - (dy - R):H, :] = 0.0
            if dx < R:
                out[:, d, :, 0:R - dx] = 0.0
            elif dx > R:
                out[:, d, :, W - (dx - R):W] = 0.0
    return out


def kernel(**inputs) -> np.ndarray:
    from concourse.bass_utils import run_bass_kernel_spmd

    f = np.ascontiguousarray(np.asarray(inputs["reference_feat"], dtype=np.float32))
    q = np.ascontiguousarray(np.asarray(inputs["query_feat"], dtype=np.float32))
    gain = float(np.asarray(inputs["init_gain"]).reshape(-1)[0])

    nc = _get_nc()
    in_maps = make_in_maps(f, q, gain)
    res = run_bass_kernel_spmd(nc, in_maps, core_ids=list(range(NCORES)))

    O = np.stack([res.results[c]["out"] for c in range(NCORES)])
    if SLICED:
        O = O.reshape(B, 2, BY, BX, NYC * NYSUB, ND, 8, WX)
    else:
        O = O.reshape(B, NYC, C, NYSUB, 8, NWIN)
    out = _extract(O)
    _clip_corrections(f, q, gain, out)
    return out


# revision 32
# speedup vs baseline: 1.1069x; 1.1069x over previous
"""LocalGOCor (PWC-Net local correlation, radius 4) on 8 Trainium2 NeuronCores.

scores[b, d, y, x] = gain * sum_c f[b,c,y,x] * q_zeropad[b, c, y+dy, x+dx]
for d = dy*9+dx, dy/dx in [0,9)  (displacement dy-4, dx-4).

v2 strategy (data-parallel over batch, 2 samples per core). The kernel is
HBM-bandwidth-bound, so every stage is built to minimize DMA bytes:

  - Inputs are quantized to int8 on the host (scale 31.75 ~ clip at 4
    sigma); on-device DVE upcasts them to bf16 holding EXACT integers in
    [-127,127], so the TensorE matmuls and fp32 PSUM accumulation are
    integer-exact (products <= 127^2, sums < 2^24).  The ~4k clipped
    elements are corrected exactly on the host.  Input HBM traffic halves
    vs bf16 (16.8 -> 8.4 MB/core).
  - Image tiled into 8x8 pixel blocks.  Per block one TensorE matmul:
    lhsT = F[c, 64 pixels], rhs = Q window [c, <=16y, <=16x] read from a
    whole-sample Q tile.  PSUM[p=(ph,ys,xs), (m, wy, wx)] for a whole
    64-row-strip of 16 blocks lives in one 4-bank tile; two blocks share
    each PE column half via tile_position (0,0)/(0,64).
  - Drains (PSUM f32 -> int8, x runtime scale c = gain*127/(75*S^2)) are
    monolithic per strip (full 128-partition efficiency) but write an
    [s, wy, m, wx]-ordered SBUF tile via a stride-permuted AP.  The
    output DMA then ships, per (ph, ys) partition group, only window
    rows [ys, ys+9) - 144 B/pixel instead of 256 - in contiguous 1152-B
    runs.  Output HBM traffic: 8.39 -> 4.72 MB/core.
  - Drain engine split ACT:DVE = 26:6 strips balances ACT (drain-only)
    against DVE (drains + all int8->bf16 upcasts at 2x).  Output DMAs go
    on the otherwise-idle GPSIMD/SWDGE queue so they never block the
    HWDGE input rings (q on SP, f on ACT).
  - Host unshard: zero-copy as_strided shear + descale + sparse exact
    clip corrections + zeroing of out-of-image displacement stripes.
"""

import numpy as np

B, C, H, W = 16, 128, 128, 128
R = 4
ND = 2 * R + 1            # 9 displacements per axis
NCORES = 8
BLOC = B // NCORES        # 2 samples per core
BY, BX = 8, 8             # pixels per block -> M = 64
WY, WX = BY + 2 * R, BX + 2 * R   # 16, 16 query window
NWIN = WY * WX            # 256
YBLK = 64                 # image rows per output chunk
NYC = H // YBLK           # 2
NYSUB = YBLK // BY        # 8 y-strips per chunk
NXB = W // BX             # 16 x-blocks
S = np.float32(31.75)     # int8 units per 1.0 of raw f/q
ODEN = np.float32(75.0)   # int8 output covers scores in [-ODEN, ODEN]
# drain-engine split: strips [0, NA) on ACT, [NA, 8) on DVE, per chunk;
# the last chunk keeps only strip 7 on DVE so the final drain chain is short
NA_PER_CHUNK = [6, 6, 7, 7]
# Output DMAs ship only window rows [g*h, g*h+8+YGRP) for each group of
# YGRP consecutive ys values -> (8+YGRP)*16 B/pixel, 2*(8/YGRP) DMAs per
# chunk wave.  YGRP=8 is the dense window (256 B/pixel, 2 DMAs/wave).
YGRP = 4
NG = BY // YGRP           # partition groups per column half
WROW = BY + YGRP          # window rows shipped per group

_CACHE = {}


def _build():
    import concourse.bacc as bacc
    import concourse.tile as tile
    import concourse.mybir as mybir
    from contextlib import ExitStack

    nc = bacc.Bacc(
        "TRN2",
        target_bir_lowering=False,
        debug=False,
        enable_asserts=False,
        num_devices=NCORES,
    )
    f32 = mybir.dt.float32
    bf16 = mybir.dt.bfloat16
    i8 = mybir.dt.int8

    f_dram = nc.dram_tensor("f8", [BLOC, C, NYC, NYSUB * NXB, BY * BX], i8,
                            kind="ExternalInput").ap()
    q_dram = nc.dram_tensor("q8", [BLOC, C, H, W], i8, kind="ExternalInput").ap()
    c_dram = nc.dram_tensor("csc", [C, 1], f32, kind="ExternalInput").ap()
    # [b, ph, h, p'(8*YGRP), s(16), wr(8+YGRP), m, wx] : per (ph, h) DMA
    # the dst block is per-partition contiguous; s = yc*8 + y0i
    o_dram = nc.dram_tensor(
        "out", [BLOC, 2, NG, BY * YGRP, NYC * NYSUB, WROW, 8, WX], i8,
        kind="ExternalOutput").ap()

    # q load pieces (rows): small leading pieces fill the pipeline fast
    QP = [(0, 12), (12, 40), (40, 68), (68, 96), (96, H)]
    FP = [(0, 16), (16, 72), (72, 128)]     # f load pieces (block-rows);
    # first piece covers exactly strip 0 so the pipeline starts fast

    with tile.TileContext(nc) as tc, ExitStack() as ctx:
        qspool = ctx.enter_context(tc.tile_pool(name="qspool", bufs=1))
        qpool = ctx.enter_context(tc.tile_pool(name="qpool", bufs=2))
        fspool = ctx.enter_context(tc.tile_pool(name="fspool", bufs=2))
        fpool = ctx.enter_context(tc.tile_pool(name="fpool", bufs=2))
        opool = ctx.enter_context(tc.tile_pool(name="opool", bufs=2))
        cpool = ctx.enter_context(tc.tile_pool(name="cpool", bufs=1))
        pspool = ctx.enter_context(tc.tile_pool(name="pspool", bufs=2, space="PSUM"))

        ct = cpool.tile([C, 1], f32, tag="ct")
        nc.sync.dma_start(out=ct[:, :], in_=c_dram[:, :])
        c_ap = ct[:, 0:1]

        # ---- input staging helpers ------------------------------------
        def load_q_piece(b, ql, pi):
            lo, hi = QP[pi]
            qs = qspool.tile([C, hi - lo, W], i8, tag=f"qs{pi}")
            nc.sync.dma_start(out=qs[:, :, :], in_=q_dram[b, :, lo:hi, :])
            return qs

        def up_q_piece(ql, qs, pi):
            lo, hi = QP[pi]
            nc.vector.tensor_scalar_mul(ql[:, lo:hi, :], qs[:, :, :], 1.0)

        def load_f(b, yc):
            fs = fspool.tile([C, NYSUB * NXB, BY * BX], i8, tag="fs")
            for lo, hi in FP:
                nc.scalar.dma_start(out=fs[:, lo:hi, :],
                                    in_=f_dram[b, :, yc, lo:hi, :])
            return fs

        def up_f(fs):
            ft = fpool.tile([C, NYSUB * NXB, BY * BX], bf16, tag="ft")
            for lo, hi in FP:
                nc.vector.tensor_scalar_mul(ft[:, lo:hi, :], fs[:, lo:hi, :], 1.0)
            return ft

        # ---- preamble -------------------------------------------------
        ql0 = qpool.tile([C, H, W], bf16, tag="ql")
        qstage0 = [load_q_piece(0, ql0, pi) for pi in range(len(QP))]
        up_q_piece(ql0, qstage0[0], 0)
        up_q_piece(ql0, qstage0[1], 1)
        chunks = [(b, yc) for b in range(BLOC) for yc in range(NYC)]
        fs_next = load_f(*chunks[0])
        ft_next = up_f(fs_next)
        up_q_piece(ql0, qstage0[2], 2)
        qls = [ql0]

        ot = None
        for ci, (b, yc) in enumerate(chunks):
            ql = qls[b]
            ft = ft_next
            na = NA_PER_CHUNK[ci]
            if yc == 0:
                # one shared output tile per sample; ACT and DVE drain
                # disjoint slots, one DMA range covers both
                ot = opool.tile([C, NYC * NYSUB, WY, 8, WX], i8, tag="ot")

            for y0i in range(NYSUB):
                r_lo = yc * YBLK + y0i * BY - R
                rl, rh = max(r_lo, 0), min(r_lo + WY, H)
                py = rl - r_lo
                pt = pspool.tile([C, 8, WY, WX], f32, tag="pt")
                for m in range(8):
                    for ph in range(2):
                        jx = ph * 8 + m
                        blk = y0i * NXB + jx
                        c_lo = BX * jx - R
                        cl, ch = max(c_lo, 0), min(c_lo + WX, W)
                        px = cl - c_lo
                        nc.tensor.matmul(
                            pt[64 * ph:64 * ph + 64, m,
                               py:py + (rh - rl), px:px + (ch - cl)],
                            ft[:, blk, :],
                            ql[:, rl:rh, cl:ch],
                            start=True, stop=True,
                            tile_position=(0, 64 * ph),
                        )

                # schedule hooks: prefetch next chunk / next sample
                if y0i == 0 and ci + 1 < len(chunks):
                    fs_next = load_f(*chunks[ci + 1])
                    ft_next = up_f(fs_next)
                    if ci == 0:
                        up_q_piece(ql, qstage0[3], 3)
                        up_q_piece(ql, qstage0[4], 4)
                if ci == 0 and y0i == 1:
                    # all q issues precede the first output DMA so the SP
                    # HWDGE ring (FIFO) never blocks an input behind a wave
                    ql1 = qpool.tile([C, H, W], bf16, tag="ql")
                    qstage1 = [load_q_piece(1, ql1, pi)
                               for pi in range(len(QP))]
                    up_q_piece(ql1, qstage1[0], 0)
                    up_q_piece(ql1, qstage1[1], 1)
                    up_q_piece(ql1, qstage1[2], 2)
                    qls.append(ql1)
                if ci == 0 and y0i == 4:
                    up_q_piece(qls[1], qstage1[3], 3)
                    up_q_piece(qls[1], qstage1[4], 4)

                # drain: PSUM f32 -> int8 with runtime scale, into the
                # [s, wy, m, wx]-ordered tile via a permuted-stride AP
                slot = NYSUB * yc + y0i
                dst = ot[:, slot, :, :, :].transpose([0, 2, 1, 3])
                if y0i < na:
                    nc.scalar.mul(dst, pt[:, :, :, :], c_ap)
                else:
                    nc.vector.tensor_scalar_mul(dst, pt[:, :, :, :], c_ap)

                # output DMA waves on the SP HWDGE ring (cheap RTL
                # desc-gen; SWDGE costs ~1us Pool seq time per dma_start);
                # half-chunk waves overlap shipping with compute, and the
                # last chunk tapers to 2-slot waves to shrink the end tail
                last = ci == len(chunks) - 1
                s_base = NYSUB * yc
                waves = []
                if y0i == 3:
                    waves.append((s_base, s_base + 4))
                elif not last and y0i == NYSUB - 1:
                    waves.append((s_base + 4, s_base + NYSUB))
                elif last and y0i == 5:
                    waves.append((s_base + 4, s_base + 6))
                elif last and y0i == NYSUB - 1:
                    waves.append((s_base + 6, s_base + NYSUB))
                for s0, s1 in waves:
                    for ph in range(2):
                        for h in range(NG):
                            p0 = 64 * ph + 8 * YGRP * h
                            r0 = YGRP * h
                            nc.sync.dma_start(
                                out=o_dram[b, ph, h, :, s0:s1],
                                in_=ot[p0:p0 + 8 * YGRP, s0:s1,
                                       r0:r0 + WROW, :, :])

    nc.compile()
    return nc


def _get_nc():
    if "nc" not in _CACHE:
        _CACHE["nc"] = _build()
    return _CACHE["nc"]


def _quant(x):
    r = np.rint(np.asarray(x, dtype=np.float32) * S)
    return np.clip(r, -127, 127).astype(np.int8)


def pack_f8(f):
    """[Bany, C, H, W] f32 -> int8 [Bany, C, NYC, NYSUB*NXB, BY*BX]."""
    v = _quant(f)
    n = v.shape[0]
    v = v.reshape(n, C, NYC, NYSUB, BY, NXB, BX)
    v = v.transpose(0, 1, 2, 3, 5, 4, 6)
    return np.ascontiguousarray(v.reshape(n, C, NYC, NYSUB * NXB, BY * BX))


def make_in_maps(f, q, gain):
    fp = pack_f8(f)
    qb = _quant(q)
    csc = np.full((C, 1), gain * 127.0 / (ODEN * S * S), dtype=np.float32)
    return [
        {"f8": fp[BLOC * c:BLOC * (c + 1)], "q8": qb[BLOC * c:BLOC * (c + 1)],
         "csc": csc}
        for c in range(NCORES)
    ]


def _clip_corrections(f, q, gain, est):
    """Add exact corrections for host-clipped int8 elements into est."""
    fr = np.rint(f * S)
    qr = np.rint(q * S)
    fcl = np.abs(fr) > 127
    qcl = np.abs(qr) > 127
    if not (fcl.any() or qcl.any()):
        return
    f8 = np.clip(fr, -127, 127) * (1.0 / S)
    q8 = np.clip(qr, -127, 127) * (1.0 / S)
    g = np.float32(gain)
    h, w = f.shape[2], f.shape[3]
    bi, ci, yi, xi = np.nonzero(fcl)
    qpad_t = np.pad(q, ((0, 0), (0, 0), (R, R), (R, R)))
    qpad_8 = np.pad(q8, ((0, 0), (0, 0), (R, R), (R, R)))
    for dy in range(ND):
        for dx in range(ND):
            d = dy * ND + dx
            tq = qpad_t[bi, ci, yi + dy, xi + dx]
            t8 = qpad_8[bi, ci, yi + dy, xi + dx]
            delta = f[bi, ci, yi, xi] * tq - f8[bi, ci, yi, xi] * t8
            np.add.at(est, (bi, np.full_like(bi, d), yi, xi), g * delta)
    bj, cj, yj, xj = np.nonzero(qcl)
    fpad_t = np.pad(f, ((0, 0), (0, 0), (R, R), (R, R)))
    fpad_8 = np.pad(f8, ((0, 0), (0, 0), (R, R), (R, R)))
    fpad_cl = np.pad(fcl, ((0, 0), (0, 0), (R, R), (R, R)))
    for dy in range(ND):
        for dx in range(ND):
            d = dy * ND + dx
            tf = fpad_t[bj, cj, yj + 2 * R - dy, xj + 2 * R - dx]
            t8 = fpad_8[bj, cj, yj + 2 * R - dy, xj + 2 * R - dx]
            already = fpad_cl[bj, cj, yj + 2 * R - dy, xj + 2 * R - dx]
            delta = np.where(already, 0.0,
                             tf * q[bj, cj, yj, xj] - t8 * q8[bj, cj, yj, xj])
            yy = yj + R - dy
            xx = xj + R - dx
            ok = (yy >= 0) & (yy < h) & (xx >= 0) & (xx < w)
            np.add.at(est, (bj[ok], np.full_like(bj[ok], d), yy[ok], xx[ok]),
                      (g * delta)[ok])


def _extract(O):
    """Device int8 output -> [B, 81, H, W] f32 (band shear via as_strided).

    O: [B, 2, NG, YGRP, BX, NYC*NYSUB, WROW, 8, WX]; pixel (ys=YGRP*h+ys',
    xs) window row r (absolute YGRP*h+r) holds displacement dy = r-ys'.
    """
    Of = np.ascontiguousarray(O.astype(np.float32) * np.float32(ODEN / 127.0))
    sb, sph, sh, sys, sxs, ss, swr, sm, swx = Of.strides
    T = np.lib.stride_tricks.as_strided(
        Of,
        shape=(B, ND, ND, NYC * NYSUB, NG, YGRP, 2, 8, BX),
        strides=(sb, swr, swx, ss, sh, sys + swr, sph, sm, sxs + swx),
    )
    out = np.ascontiguousarray(T.reshape(B, ND * ND, H, W))
    for dy in range(ND):
        for dx in range(ND):
            d = dy * ND + dx
            if dy < R:
                out[:, d, 0:R - dy, :] = 0.0
            elif dy > R:
                out[:, d, H - (dy - R):H, :] = 0.0
            if dx < R:
                out[:, d, :, 0:R - dx] = 0.0
            elif dx > R:
                out[:, d, :, W - (dx - R):W] = 0.0
    return out


def kernel(**inputs) -> np.ndarray:
    from concourse.bass_utils import run_bass_kernel_spmd

    f = np.ascontiguousarray(np.asarray(inputs["reference_feat"], dtype=np.float32))
    q = np.ascontiguousarray(np.asarray(inputs["query_feat"], dtype=np.float32))
    gain = float(np.asarray(inputs["init_gain"]).reshape(-1)[0])

    nc = _get_nc()
    in_maps = make_in_maps(f, q, gain)
    res = run_bass_kernel_spmd(nc, in_maps, core_ids=list(range(NCORES)))

    O = np.stack([res.results[c]["out"] for c in range(NCORES)])
    O = O.reshape(B, 2, NG, YGRP, BX, NYC * NYSUB, WROW, 8, WX)
    out = _extract(O)
    _clip_corrections(f, q, gain, out)
    return out
